# revision 37
# baseline (speedup 1.0000x reference)
"""Trainium2 Bass kernel for DigitConvolutionalModel (conv3x3 + 4-layer MLP).

Strategy:
  - Conv folds into W1 on host: W1eff[784,1024] = C @ W1.T; device runs a pure
    4-layer MLP. Pure data parallelism: batch 16384 -> 2048 rows per core.
  - Feature-major on device: h[features, batch]; each layer out = W.T @ h with
    W k-tiles stationary, batch (N=512) moving. bf16 inputs, fp32 PSUM.
  - DMA: one dma_start distributes its packets round-robin over all 16 HWDGE
    queues (~300GB/s aggregate), so inputs ship as ~17 chunked DMAs issued
    from the main block (pre tile-context) on both HWDGE rings instead of
    ~55 serialized small ones. Transfers complete in roughly GLOBAL issue
    order (the rings share the 16 DMA engines), so the issue order is
    arranged to match the L1 k-progression, with bt2/bt3 x at the tail of
    the scalar ring behind the ramp-critical w1 chunks.
  - Completion tracking: one semaphore PER chunk (a chunk's "+16" arrives as
    16 per-queue "+1"s, so a shared cumulative sem can pass early). Consumer
    instructions get their waits attached AFTER tile scheduling (the
    scheduler's internal sim would deadlock on sems incremented from the
    main block); bacc splits multi-waits into Ldweights/EventSemaphore
    preludes. Sems are range-cleared by the waiting engines (PE/ACT) at
    stream start and again at kernel end -- values persist across NEFF
    executions and stale values caused first-run corruption otherwise.
  - L1 K=784 = 6 full k-tiles + a 16-row tail. The tail is row-packed: x rows
    768:784 are replicated at partition bases 0/32/64/96 and the 8 m-tiles'
    k6 weights run as 2 quads of 4 concurrent K=16 matmuls (tile_position),
    saving ~6 matmul slots/bt (no zero-padding, no memsets). The quads run
    before the k5 step so the k5 stop releases each m's relu individually.
  - Relus alternate ScalarE / DVE by m parity (different PSUM banks run in
    parallel) with the per-partition bias fused (Relu bias= / add+max).
  - L4 [256->10]: the first bt pair runs in two PE column groups
    (tile_position=(0,32)) concurrently; b4 is added by the ScalarE
    Identity-activation copy (bias AP), not a ones-row matmul.
  - 8 warmup matmuls on garbage SBUF right after the entry barrier hold the
    PE busy from ~7.2us (barrier) to ~10.6us (first data) so the HAM clock
    gate (1.2 -> 2.4 GHz) releases before real work and never re-throttles.
  - Fixed costs: ~6.9us engine preamble + entry barrier before any user
    instruction can run, ~2.3us exit barrier/drain; first DMA data cannot
    land before ~8.5us (issue + DGE latency). Measured ~96.9us vs the
    ~105.6us baseline; the matmul stream itself is issue-bound at
    ~216ns per 128x128x512 bf16 matmul.
"""

import numpy as np
import ml_dtypes
from contextlib import ExitStack

import concourse.mybir as mybir
import concourse.tile as tile
from concourse import bacc
from concourse.bass_utils import run_bass_kernel_spmd

F32 = mybir.dt.float32
BF16 = mybir.dt.bfloat16
AF = mybir.ActivationFunctionType
NP_BF16 = ml_dtypes.bfloat16

N_CORES = 8
B = 16384
BC = B // N_CORES          # 2048 rows per core
BT = 512                   # batch tile (matmul free dim; PSUM bank = 512 fp32)
NBT = BC // BT             # 4
K1 = 784                   # 28*28 (conv folded into W1)
D1, D2, D3, D4 = 1024, 512, 256, 10
XCB = 7 * BT               # x columns per bt block (6 k-tiles + k6 block)

# w_d column layout (bf16):  w1 k-blocks | k6 pack | w2 | w3 | w4
W1_OFF = 0                  # 6 full k-tiles of [128, 1024]
K6_OFF = 6 * D1             # [128, 256]: partitions 32g+0:16 hold w1e rows 768+r,
                            # cols 128q:128(q+1) for m = 4q+g
W2_OFF = K6_OFF + 256       # 8 blocks of [128, 512]
W3_OFF = W2_OFF + 8 * D2    # 4 blocks of [128, 256]
W4_OFF = W3_OFF + 4 * D3    # 2 blocks of [128, 10]
W_COLS = W4_OFF + 2 * D4

N_WARM = 8                  # garbage warmups bridging barrier -> first data

# SP-ring chunks of x_d (+bias separately) and ACT-ring chunks of w_d.
# L1 runs k-outer over all 8 m-tiles (one k-tile lasts ~1.7us), so chunks need
# only keep one k-step ahead of a ~1.7us/k consumption cadence.
SP_CHUNKS = [               # (start_col, end_col) into x_d; None = bias
    (0 * BT, 1 * BT),       # S1: bt0 k0
    (1 * BT, 2 * BT),       # S2: bt0 k1
    (2 * BT, 3 * BT),       # S3: bt0 k2
    (3 * BT, 5 * BT),       # S4: bt0 k3,k4
    None,                   # S5: bias
    (5 * BT, XCB),          # S6: bt0 k5,k6
    (XCB, XCB + 3 * BT),    # S7: bt1 k0-k2
    (XCB + 3 * BT, 2 * XCB),  # S8: bt1 k3-k6
]
# bt2/bt3 x ride at the TAIL of the scalar ring so their big transfers queue
# behind the ramp-critical w1 chunks (transfer order ~= global issue order).
ACT_XTRA = [
    (2 * XCB, 3 * XCB),     # bt2  (chunk idx 15)
    (3 * XCB, 4 * XCB),     # bt3  (chunk idx 17)
]
ACT_CHUNKS = [              # (start_col, end_col) into w_d
    (0, D1),                # A1: w1 k0
    (D1, 2 * D1),           # A2: w1 k1
    (2 * D1, 3 * D1),       # A3: w1 k2
    (3 * D1, 4 * D1),       # A4: w1 k3
    (4 * D1, 5 * D1),       # A5: w1 k4
    (5 * D1, W2_OFF),       # A6: w1 k5 + k6pack
    (W3_OFF, W_COLS),       # A7: w3,w4
    None,                   # A8: x bt2   (ACT_XTRA[0], chunk idx 15)
    (W2_OFF, W3_OFF),       # A9: w2
    None,                   # A10: x bt3  (ACT_XTRA[1], chunk idx 17)
]


# chunk index (into the 13 per-chunk semaphores: SP chunks 0-6, ACT 7-12)
def _sp_chunk(bt, k):
    if bt == 1:
        return 6 if k <= 2 else 7
    if bt > 1:
        return 15 if bt == 2 else 17
    if k <= 2:
        return k
    return 3 if k <= 4 else 5


def _act_chunk_l1(k, m):
    return 8 + min(k, 5)   # A1..A6; the k6 quad rides with k5's chunk


def _build_nc():
    nc = bacc.Bacc(None)

    x_d = nc.dram_tensor("x", [128, NBT * XCB], BF16, kind="ExternalInput")
    w_d = nc.dram_tensor("w", [128, W_COLS], BF16, kind="ExternalInput")
    bias_d = nc.dram_tensor("bias", [128, 16], F32, kind="ExternalInput")
    out_d = nc.dram_tensor("out", [D4, BC], F32, kind="ExternalOutput")

    es = ExitStack()
    x_sb = es.enter_context(nc.sbuf_tensor("x_sb", [128, NBT * XCB], BF16))
    w_sb = es.enter_context(nc.sbuf_tensor("w_sb", [128, W_COLS], BF16))
    bias_sb = es.enter_context(nc.sbuf_tensor("bias_sb2", [128, 16], F32))
    warm_sb = es.enter_context(nc.sbuf_tensor("warm_sb2", [128, 640], BF16))

    # One semaphore PER DMA chunk (like the framework's rotating DMAHW sems).
    # A chunk's "+16" lands as 16 per-queue "+1"s, so a shared cumulative sem
    # would let `>= 16*n` pass while an earlier chunk is still in flight on
    # the slower queues -- seen as one-core bt0 corruption on first runs.
    sems = [nc.alloc_semaphore(f"dma_ch{i}") for i in range(18)]

    # ---- main block (pre tile-context): warmups + all input DMA issues ----
    # Sem hygiene: the NEFF executes more than once per load (profiling
    # warm-up runs) and sem values persist across executions. The waiting
    # engines (PE, ACT) clear all chunk sems as their first post-barrier
    # instructions -- with a >1us margin before the first completion inc can
    # arrive -- and the sems are cleared again at kernel end.
    sem_rng = range(sems[0].num, sems[0].num + len(sems))
    nc.tensor.sem_clear(sem_rng)
    nc.scalar.sem_clear(sem_rng)
    warm_ps = nc.place_psum_tensor("warm_ps", [128, BT], F32, bank=7)
    for _ in range(N_WARM):
        nc.tensor.matmul(warm_ps[:], warm_sb[:, 0:128], warm_sb[:, 128:640],
                         start=True, stop=True)
    for i, ch in enumerate(SP_CHUNKS):
        if ch is None:
            nc.sync.dma_start(out=bias_sb[:], in_=bias_d[:]).then_inc(sems[i], 16)
        else:
            nc.sync.dma_start(out=x_sb[:, ch[0]:ch[1]],
                              in_=x_d[:, ch[0]:ch[1]]).then_inc(sems[i], 16)
    xtra = iter(ACT_XTRA)
    for j, ch in enumerate(ACT_CHUNKS):
        if ch is None:
            xc = next(xtra)
            nc.scalar.dma_start(out=x_sb[:, xc[0]:xc[1]],
                                in_=x_d[:, xc[0]:xc[1]]).then_inc(sems[8 + j], 16)
        else:
            nc.scalar.dma_start(out=w_sb[:, ch[0]:ch[1]],
                                in_=w_d[:, ch[0]:ch[1]]).then_inc(sems[8 + j], 16)

    CH_BIAS = 4
    CH_W2 = 16
    CH_W34 = 14

    deferred = []           # (inst, chunk_index) applied post-scheduling

    def xk(bt, k):          # x k-tile [128, 512] (k6: replicated 16-row blocks)
        return x_sb[:, (bt * 7 + k) * BT:(bt * 7 + k + 1) * BT]

    def w1s(k, m):          # w1 k-tile m-slice [128, 128]
        return w_sb[:, k * D1 + m * 128:k * D1 + (m + 1) * 128]

    with tile.TileContext(nc) as tc, ExitStack() as ctx:
        sb = ctx.enter_context(tc.tile_pool(name="sb", bufs=1))
        psum = ctx.enter_context(tc.tile_pool(name="psum", bufs=8, space="PSUM"))

        h1 = [[sb.tile([128, BT], BF16, tag=f"h1_{m}_{bt}", name=f"h1_{m}_{bt}")
               for bt in range(NBT)] for m in range(8)]
        h2 = [[sb.tile([128, BT], BF16, tag=f"h2_{m}_{bt}", name=f"h2_{m}_{bt}")
               for bt in range(NBT)] for m in range(4)]
        h3 = [[sb.tile([128, BT], BF16, tag=f"h3_{m}_{bt}", name=f"h3_{m}_{bt}")
               for bt in range(NBT)] for m in range(2)]
        outsb = sb.tile([128, BC], F32, tag="o", name="o")

        # ---------------- layer 1: x[784, BC] -> relu h1[1024, BC] ----------
        # bt0 runs k-outer over all 8 m-tiles so each DMA chunk has a full
        # ~1.7us k-step of runway during the ramp; later bts run half-passes
        # of 4 m-tiles (relus interleave between halves, no boundary stall).
        def l1_kmm(ps, m, k, bt, start, stop=False):
            mm = nc.tensor.matmul(ps[:], w1s(k, m), xk(bt, k),
                                  start=start, stop=stop)
            deferred.append((mm, _sp_chunk(bt, k)))
            deferred.append((mm, _act_chunk_l1(k, m)))

        def l1_quad(ps, q, bt):           # 4 concurrent K=16 tail matmuls
            for g in range(4):
                m = 4 * q + g
                lo = 32 * g
                mm = nc.tensor.matmul(
                    ps[m][:],
                    w_sb[lo:lo + 16, K6_OFF + q * 128:K6_OFF + (q + 1) * 128],
                    x_sb[lo:lo + 16, (bt * 7 + 6) * BT:(bt * 7 + 7) * BT],
                    start=False, stop=False, tile_position=(lo, 0))
                deferred.append((mm, _sp_chunk(bt, 6)))
                deferred.append((mm, 13))

        def relu_any(dst, src, bias_ap, alt):
            # alternate ScalarE / DVE so relu cadence never gates PSUM reuse
            if alt:
                op = nc.vector.tensor_scalar(dst, src, bias_ap, 0.0,
                                             mybir.AluOpType.add,
                                             mybir.AluOpType.max)
            else:
                op = nc.scalar.activation(dst, src, AF.Relu, bias=bias_ap)
            deferred.append((op, CH_BIAS))

        def l1_relu(pst, m, bt):
            relu_any(h1[m][bt][:], pst[:], bias_sb[:, m:m + 1], m % 2 == 1)

        # bt0: k-outer over all 8 m-tiles
        ps0 = {m: psum.tile([128, BT], F32, tag="ps", name=f"ps1_{m}_0")
               for m in range(8)}
        for k in range(5):
            for m in range(8):
                l1_kmm(ps0[m], m, k, 0, start=(k == 0))
        l1_quad(ps0, 0, 0)
        l1_quad(ps0, 1, 0)
        for m in range(8):           # k5 carries the stop; relu m fires per-m
            l1_kmm(ps0[m], m, 5, 0, start=False, stop=True)
            l1_relu(ps0[m], m, 0)

        # bt1..3: two half-passes of 4 m-tiles each
        for bt in range(1, NBT):
            for q in range(2):
                ms = range(4 * q, 4 * q + 4)
                ps = {m: psum.tile([128, BT], F32, tag="ps", name=f"ps1_{m}_{bt}")
                      for m in ms}
                for k in range(5):
                    for m in ms:
                        l1_kmm(ps[m], m, k, bt, start=(k == 0))
                l1_quad(ps, q, bt)
                for m in ms:
                    l1_kmm(ps[m], m, 5, bt, start=False, stop=True)
                    l1_relu(ps[m], m, bt)

        # ---------------- layer 2: [1024] -> relu h2[512] ----------
        for bt in range(NBT):
            for m in range(4):
                p = psum.tile([128, BT], F32, tag="ps", name=f"ps2_{m}_{bt}")
                for k in range(8):
                    mm = nc.tensor.matmul(
                        p[:], w_sb[:, W2_OFF + k * D2 + m * 128:
                                   W2_OFF + k * D2 + (m + 1) * 128],
                        h1[k][bt][:], start=(k == 0), stop=(k == 7))
                    if k == 0:
                        deferred.append((mm, CH_W2))
                relu_any(h2[m][bt][:], p[:], bias_sb[:, 8 + m:9 + m], m % 2 == 1)

        def l3(bt):
            for m in range(2):
                p = psum.tile([128, BT], F32, tag="ps", name=f"ps3_{m}_{bt}")
                for k in range(4):
                    mm = nc.tensor.matmul(
                        p[:], w_sb[:, W3_OFF + k * D3 + m * 128:
                                   W3_OFF + k * D3 + (m + 1) * 128],
                        h2[k][bt][:], start=(k == 0), stop=(k == 3))
                    if k == 0:
                        deferred.append((mm, CH_W34))
                relu_any(h3[m][bt][:], p[:], bias_sb[:, 12 + m:13 + m], m % 2 == 1)

        def l4_copy(bt, p, lo):
            # a [10, 512] ScalarE op costs the same as a half (fixed overhead)
            act = nc.scalar.activation(outsb[lo:lo + 10, bt * BT:(bt + 1) * BT],
                                       p[lo:lo + 10, :], AF.Identity,
                                       bias=bias_sb[lo:lo + 10, 14:15])
            deferred.append((act, CH_BIAS))

        def l4_pair(bta, btb):
            # two bts in different PE column groups; k-tiles accumulate.
            pa = psum.tile([128, BT], F32, tag="ps", name=f"ps4_{bta}")
            pb = psum.tile([128, BT], F32, tag="ps", name=f"ps4_{btb}")
            for kt in range(2):
                wsl = w_sb[:, W4_OFF + kt * D4:W4_OFF + (kt + 1) * D4]
                mm = nc.tensor.matmul(pa[0:10, :], wsl, h3[kt][bta][:],
                                      start=(kt == 0), stop=(kt == 1),
                                      tile_position=(0, 0))
                mmb = nc.tensor.matmul(pb[32:42, :], wsl, h3[kt][btb][:],
                                       start=(kt == 0), stop=(kt == 1),
                                       tile_position=(0, 32))
                if kt == 0:
                    deferred.append((mm, CH_W34))
                    deferred.append((mmb, CH_W34))
            for bt, p, lo in ((bta, pa, 0), (btb, pb, 32)):
                l4_copy(bt, p, lo)
                nc.sync.dma_start(out=out_d[:, bt * BT:(bt + 1) * BT],
                                  in_=outsb[lo:lo + 10, bt * BT:(bt + 1) * BT])

        def l4_solo(bt, split):
            cols = slice(bt * BT, (bt + 1) * BT)
            if not split:
                p = psum.tile([128, BT], F32, tag="ps", name=f"ps4_{bt}")
                for kt in range(2):
                    wsl = w_sb[:, W4_OFF + kt * D4:W4_OFF + (kt + 1) * D4]
                    mm = nc.tensor.matmul(p[0:10, :], wsl, h3[kt][bt][:],
                                          start=(kt == 0), stop=(kt == 1),
                                          tile_position=(0, 0))
                    if kt == 0:
                        deferred.append((mm, CH_W34))
                l4_copy(bt, p, 0)
                nc.sync.dma_start(out=out_d[:, cols], in_=outsb[0:10, cols])
                return
            # tail bt: same as unsplit (kept for clarity of the call sites)
            p = psum.tile([128, BT], F32, tag="ps", name=f"ps4_{bt}")
            for kt in range(2):
                wsl = w_sb[:, W4_OFF + kt * D4:W4_OFF + (kt + 1) * D4]
                mm = nc.tensor.matmul(p[0:10, :], wsl, h3[kt][bt][:],
                                      start=(kt == 0), stop=(kt == 1),
                                      tile_position=(0, 0))
                if kt == 0:
                    deferred.append((mm, CH_W34))
            l4_copy(bt, p, 0)
            nc.sync.dma_start(out=out_d[:, cols], in_=outsb[0:10, cols])

        # layer 3/4 interleaved so the last output DMA starts as early as it can
        l3(0); l3(1); l3(2); l3(3)
        l4_pair(0, 1)
        l4_solo(2, split=False)
        l4_solo(3, split=True)

    # end-of-kernel sem hygiene: leave the DMA sems at 0 for the next run
    nc.sync.sem_clear(sem_rng)

    for inst, chunk in deferred:
        inst.wait_op(sems[chunk], 16, "sem-ge", check=False)
    nc.finalize()
    es.close()
    return nc


def _fold_conv(conv_w, W1):
    """W1eff[784,1024] such that x @ W1eff == conv3x3(x, conv_w) @ W1.T."""
    W1img = W1.reshape(D1, 26, 26).transpose(1, 2, 0).astype(np.float32)
    W1e = np.zeros((28, 28, D1), np.float32)
    for di in range(3):
        for dj in range(3):
            W1e[di:di + 26, dj:dj + 26, :] += np.float32(conv_w[di, dj]) * W1img
    return W1e.reshape(K1, D1)


def _prep_inputs(inputs):
    x = np.asarray(inputs["x"], np.float32)
    conv_w = np.asarray(inputs["conv_w"], np.float32)
    W1 = np.asarray(inputs["W1"], np.float32)
    b1 = np.asarray(inputs["b1"], np.float32)
    W2 = np.asarray(inputs["W2"], np.float32)
    b2 = np.asarray(inputs["b2"], np.float32)
    W3 = np.asarray(inputs["W3"], np.float32)
    b3 = np.asarray(inputs["b3"], np.float32)
    W4 = np.asarray(inputs["W4"], np.float32)
    b4 = np.asarray(inputs["b4"], np.float32)

    w1e = _fold_conv(conv_w, W1)                        # [784, 1024] fp32

    wpack = np.zeros((128, W_COLS), np.float32)
    for k in range(6):                                  # w1 full k-tiles
        wpack[:, k * D1:(k + 1) * D1] = w1e[k * 128:(k + 1) * 128]
    for q in range(2):                                  # k6 pack (rows 768:784)
        for g in range(4):
            m = 4 * q + g
            wpack[32 * g:32 * g + 16, K6_OFF + q * 128:K6_OFF + (q + 1) * 128] \
                = w1e[768:784, m * 128:(m + 1) * 128]
    w2 = np.ascontiguousarray(W2.T)                     # [1024, 512]
    for k in range(8):
        wpack[:, W2_OFF + k * D2:W2_OFF + (k + 1) * D2] = w2[k * 128:(k + 1) * 128]
    w3 = np.ascontiguousarray(W3.T)                     # [512, 256]
    for k in range(4):
        wpack[:, W3_OFF + k * D3:W3_OFF + (k + 1) * D3] = w3[k * 128:(k + 1) * 128]
    w4 = np.ascontiguousarray(W4.T)                     # [256, 10]
    for k in range(2):
        wpack[:, W4_OFF + k * D4:W4_OFF + (k + 1) * D4] = w4[k * 128:(k + 1) * 128]
    wpack = wpack.astype(NP_BF16)

    bias_pack = np.zeros((128, 16), np.float32)
    bias_pack[:, 0:8] = b1.reshape(8, 128).T
    bias_pack[:, 8:12] = b2.reshape(4, 128).T
    bias_pack[:, 12:14] = b3.reshape(2, 128).T
    bias_pack[0:10, 14] = b4
    bias_pack[32:42, 14] = b4

    in_maps = []
    for c in range(N_CORES):
        xs = np.ascontiguousarray(x[c * BC:(c + 1) * BC].T)  # [784, 2048] fp32
        xh = np.zeros((128, NBT * XCB), np.float32)
        for bt in range(NBT):
            cols = slice(bt * BT, (bt + 1) * BT)
            for k in range(6):
                xh[:, (bt * 7 + k) * BT:(bt * 7 + k + 1) * BT] = \
                    xs[k * 128:(k + 1) * 128, cols]
            for g in range(4):
                xh[32 * g:32 * g + 16, (bt * 7 + 6) * BT:(bt * 7 + 7) * BT] = \
                    xs[768:784, cols]
        in_maps.append({"x": xh.astype(NP_BF16), "w": wpack, "bias": bias_pack})
    return in_maps


def _run(inputs, trace=False):
    nc = _build_nc()
    in_maps = _prep_inputs(inputs)
    res = run_bass_kernel_spmd(nc, in_maps, core_ids=list(range(N_CORES)),
                               trace=trace)
    parts = [np.asarray(r["out"], np.float32).T for r in res.results]
    out = np.concatenate(parts, axis=0)                 # [16384, 10]
    return out, res


def kernel(**inputs):
    out, _ = _run(inputs, trace=False)
    return out
